# revision 1
# baseline (speedup 1.0000x reference)
"""CrossAttention (DFFNet) Trainium2 Bass kernel.

Shapes (hardcoded): rgb/depth [4, 256, 64, 64] f32; Wq/Wk [32, 256]; Wv [256, 256].

    q = Wq @ d + bq          [B, 32, 4096]
    k = Wk @ d + bk          [B, 32, 4096]
    v = Wv @ r + bv          [B, 256, 4096]
    scores = q^T k           [B, 4096, 4096], softmax over keys (last dim)
    feat = v @ mask^T        [B, 256, 4096]

Sharding: 8 cores = 4 batches x 2 query-halves (2048 queries each). Each core
gets full depth/rgb for its batch (keys/values span all 4096 tokens) plus its
query-half slice of depth.

Device layout choice: scores are computed TRANSPOSED, st[m, n] (keys m on
partitions, queries n free) so the feat matmul needs no transposes:
  - v^T[m, c] is produced directly by  r-slice^T @ Wv^T  (r already has
    channels on partitions, which is the contraction dim).
  - feat[c, n] = sum_m v^T[m, c] * exp(st[m, n]) / S[n]  -> lhsT = v^T tile,
    rhs = exp(st) tile, both with m on partitions.
  - softmax denominator S[n] = sum_m exp(st[m,n]) is a partition-axis sum ->
    ones[128,1]-lhsT matmul accumulated over m-tiles (PE streams it).
  - no max-subtraction: |scores| < ~6 here, exp is well-conditioned.
Normalization: 1/S via fast reciprocal (DVE), broadcast to 128 partitions via
a K=1 matmul with a ones row, multiply + bias-add on DVE.

The K=32 score matmuls are 4-way row-packed (tile_position=(32j, 0)): k and q
are kept in 4x-replicated layouts [128, *] (4 copies at partition offsets
0/32/64/96), which fall out of the projection matmuls for free by tiling the
tiny weight matrices host-side (WkT_4x = tile(Wk.T, (1, 4))).

All matmuls run as float32r (full PE rate at free-dim >= 256, fp32 storage).
"""

import numpy as np

import concourse.bacc as bacc
import concourse.bass as bass
import concourse.mybir as mybir
import concourse.tile as tile
from concourse.bass_utils import run_bass_kernel_spmd

B, C, H, W = 4, 256, 64, 64
HW = H * W            # 4096
CQK = 32
P = 128
NQ = HW // 2          # 2048 queries per core
NT = 512              # query tile
N_NT = NQ // NT       # 4
MT = HW // P          # 32 key tiles
KC = C // P           # 2 contraction tiles for the projections

F32 = mybir.dt.float32
F32R = mybir.dt.float32r
BF16 = mybir.dt.bfloat16
AF = mybir.ActivationFunctionType
OP = mybir.AluOpType


def _r(ap):
    """View an fp32 AP as float32r (valid only after _round_inplace)."""
    return ap.bitcast(F32R)


def _staged_load(nc, pool, dst, dram_ap, chunk=1024):
    """DMA fp32 DRAM -> small staging tile, DVE-copy (rounding) -> f32r dst."""
    n = dst.shape[1]
    for c0 in range(0, n, chunk):
        w = min(chunk, n - c0)
        stg = pool.tile([P, chunk], F32, tag="stage", name=f"stg_{dst.name}_{c0}")
        nc.sync.dma_start(stg[:, 0:w], dram_ap[:, c0:c0 + w])
        nc.vector.tensor_copy(dst[:, c0:c0 + w], stg[:, 0:w])


def _emit(tc, io):
    nc = tc.nc
    d = io["d"].ap()          # [256, 4096] depth (keys source)
    dq = io["dq"].ap()        # [256, 2048] depth query-half
    r = io["r"].ap()          # [256, 4096] rgb (values source)
    wqt4 = io["wqt4"].ap()    # [256, 128] = tile(Wq.T, (1,4))
    wkt4 = io["wkt4"].ap()    # [256, 128]
    wvt = io["wvt"].ap()      # [256, 256] = Wv.T
    bq4 = io["bq4"].ap()      # [128, 1] = tile(bq, 4)
    bk4 = io["bk4"].ap()      # [128, 1]
    bv2 = io["bv2"].ap()      # [256, 1]
    out = io["out"].ap()      # [256, 2048]

    from contextlib import ExitStack

    with ExitStack() as ctx:
        pw = ctx.enter_context(tc.tile_pool(name="weights", bufs=1))
        pin = ctx.enter_context(tc.tile_pool(name="inputs", bufs=1))
        pqk = ctx.enter_context(tc.tile_pool(name="qk", bufs=1))
        pvt = ctx.enter_context(tc.tile_pool(name="vt", bufs=1))
        pse = ctx.enter_context(tc.tile_pool(name="stexp", bufs=2))
        psmall = ctx.enter_context(tc.tile_pool(name="small", bufs=2))
        pout = ctx.enter_context(tc.tile_pool(name="outsb", bufs=4))
        pstage = ctx.enter_context(tc.tile_pool(name="stage", bufs=4))
        ps_st = ctx.enter_context(
            tc.tile_pool(name="ps_st", bufs=2, space=bass.MemorySpace.PSUM))
        ps_feat = ctx.enter_context(
            tc.tile_pool(name="ps_feat", bufs=2, space=bass.MemorySpace.PSUM))
        ps_sums = ctx.enter_context(
            tc.tile_pool(name="ps_sums", bufs=1, space=bass.MemorySpace.PSUM))
        ps_bc = ctx.enter_context(
            tc.tile_pool(name="ps_bc", bufs=1, space=bass.MemorySpace.PSUM))

        # ---- constants / weights --------------------------------------
        wq_t, wk_t, wv_t, bv_t = [], [], [], []
        for kc in range(KC):
            t = pw.tile([P, P], BF16, tag=f"wq{kc}")
            _staged_load(nc, pstage, t, wqt4[kc * P:(kc + 1) * P, :])
            wq_t.append(t)
            t = pw.tile([P, P], BF16, tag=f"wk{kc}")
            _staged_load(nc, pstage, t, wkt4[kc * P:(kc + 1) * P, :])
            wk_t.append(t)
            t = pw.tile([P, C], BF16, tag=f"wv{kc}")
            _staged_load(nc, pstage, t, wvt[kc * P:(kc + 1) * P, :])
            wv_t.append(t)
            t = pw.tile([P, 1], F32, tag=f"bv{kc}")
            nc.sync.dma_start(t[:], bv2[kc * P:(kc + 1) * P, :])
            bv_t.append(t)
        bq_sb = pw.tile([P, 1], F32, tag="bq")
        nc.sync.dma_start(bq_sb[:], bq4[:])
        bk_sb = pw.tile([P, 1], F32, tag="bk")
        nc.sync.dma_start(bk_sb[:], bk4[:])
        ones_f = pw.tile([P, 1], F32, tag="ones_f")
        nc.vector.memset(ones_f[:], 1.0)
        ones_col = pw.tile([P, 1], BF16, tag="ones_col")
        nc.vector.tensor_copy(ones_col[:], ones_f[:])
        ones_row = pw.tile([1, P], F32, tag="ones_row")
        nc.vector.memset(ones_row[:], 1.0)

        # ---- inputs (per-chunk tiles so projections start early) -------
        def _chunked(dram_ap, kc, nch, pref):
            tiles = []
            for ch in range(nch):
                t = pin.tile([P, 1024], BF16, tag=f"{pref}{kc}_{ch}",
                             name=f"{pref}{kc}_{ch}")
                stg = pstage.tile([P, 1024], F32, tag="stage",
                                  name=f"stg_{pref}{kc}_{ch}")
                nc.sync.dma_start(
                    stg[:], dram_ap[kc * P:(kc + 1) * P,
                                    ch * 1024:(ch + 1) * 1024])
                nc.vector.tensor_copy(t[:], stg[:])
                tiles.append(t)
            return tiles

        d_sb = [_chunked(d, kc, 4, "d") for kc in range(KC)]
        dq_sb = [_chunked(dq, kc, 2, "dq") for kc in range(KC)]
        r_sb = [_chunked(r, kc, 4, "r") for kc in range(KC)]

        # ---- k / q projections (4x-replicated layouts) -----------------
        # k4[32j + o, m] = k[o, m];  q4[32j + o, n] = q[o, n]
        k4 = pqk.tile([P, HW], BF16, tag="k4")
        for qtr in range(4):
            kp = ps_st.tile([P, 1024], F32, tag="stp", name=f"kp{qtr}")
            for sub in range(2):
                n0 = sub * NT
                g0 = qtr * 1024 + n0
                for kc in range(KC):
                    nc.tensor.matmul(
                        kp[:, n0:n0 + NT],
                        lhsT=wk_t[kc][:],
                        rhs=d_sb[kc][qtr][:, n0:n0 + NT],
                        start=(kc == 0),
                        stop=(kc == KC - 1),
                    )
            nc.vector.tensor_scalar(
                k4[:, qtr * 1024:(qtr + 1) * 1024], kp[:], bk_sb[:], None, OP.add
            )
        q4 = pqk.tile([P, NQ], BF16, tag="q4")
        for half in range(2):
            qp = ps_st.tile([P, 1024], F32, tag="stp", name=f"qp{half}")
            for sub in range(2):
                n0 = sub * NT
                g0 = half * 1024 + n0
                for kc in range(KC):
                    nc.tensor.matmul(
                        qp[:, n0:n0 + NT],
                        lhsT=wq_t[kc][:],
                        rhs=dq_sb[kc][half][:, n0:n0 + NT],
                        start=(kc == 0),
                        stop=(kc == KC - 1),
                    )
            nc.vector.tensor_scalar(
                q4[:, half * 1024:(half + 1) * 1024], qp[:], bq_sb[:], None, OP.add
            )

        # ---- v^T projection: vt[mt][p, c] = v[c, mt*128 + p] (no bias) --
        vt_t = []
        for mt in range(MT):
            vp = ps_feat.tile([P, C], F32, tag="feat")
            for kc in range(KC):
                nc.tensor.matmul(
                    vp[:],
                    lhsT=r_sb[kc][mt // 8][:, (mt % 8) * P:(mt % 8 + 1) * P],
                    rhs=wv_t[kc][:],
                    start=(kc == 0),
                    stop=(kc == KC - 1),
                )
            t = pvt.tile([P, C], BF16, tag=f"vt{mt}")
            nc.vector.tensor_copy(t[:], vp[:])
            vt_t.append(t)

        # ---- main attention loop ---------------------------------------
        for nt in range(N_NT):
            n0 = nt * NT
            fc = [ps_feat.tile([P, NT], F32, tag="feat", name=f"fc{nt}_{i}") for i in range(2)]
            sm = ps_sums.tile([1, NT], F32, tag="sums")
            for g in range(MT // 2):
                stp = ps_st.tile([P, 1024], F32, tag="stp", name=f"stp{nt}_{g}")
                for j in range(2):
                    mt = 2 * g + j
                    nc.tensor.matmul(
                        stp[:, j * NT:(j + 1) * NT],
                        lhsT=k4[32 * j:32 * j + 32, mt * P:(mt + 1) * P],
                        rhs=q4[32 * j:32 * j + 32, n0:n0 + NT],
                        start=True,
                        stop=True,
                        tile_position=(32 * j, 0),
                    )
                se = pse.tile([P, 1024], BF16, tag="se", name=f"se{nt}_{g}")
                nc.scalar.activation(se[:], stp[:], AF.Exp)
                for j in range(2):
                    mt = 2 * g + j
                    sej = se[:, j * NT:(j + 1) * NT]
                    first = mt == 0
                    last = mt == MT - 1
                    nc.tensor.matmul(
                        fc[0][:], lhsT=vt_t[mt][:, 0:P], rhs=sej,
                        start=first, stop=last,
                    )
                    nc.tensor.matmul(
                        fc[1][:], lhsT=vt_t[mt][:, P:C], rhs=sej,
                        start=first, stop=last,
                    )
                    nc.tensor.matmul(
                        sm[:], lhsT=ones_col[:], rhs=sej,
                        start=first, stop=last,
                    )
            rc = psmall.tile([1, NT], F32, tag="recip")
            nc.vector.reciprocal_approx_fast(out=rc[:], in_=sm[:])
            bc = ps_bc.tile([P, NT], F32, tag="bc")
            nc.tensor.matmul(
                bc[:], lhsT=ones_row[:], rhs=rc[:], start=True, stop=True
            )
            bc_sb = pout.tile([P, NT], F32, tag="bc_sb")
            nc.vector.tensor_copy(bc_sb[:], bc[:])
            for c in range(2):
                tmp = pout.tile([P, NT], F32, tag="tmp")
                nc.vector.tensor_tensor(tmp[:], fc[c][:], bc_sb[:], OP.mult)
                ot = pout.tile([P, NT], F32, tag="ot")
                nc.vector.tensor_scalar(ot[:], tmp[:], bv_t[c][:], None, OP.add)
                nc.sync.dma_start(out[c * P:(c + 1) * P, n0:n0 + NT], ot[:])


_BUILT = None


def _build():
    global _BUILT
    if _BUILT is not None:
        return _BUILT
    nc = bacc.Bacc("TRN2", target_bir_lowering=False, debug=False)
    io = {
        "d": nc.dram_tensor("d", [C, HW], F32, kind="ExternalInput"),
        "dq": nc.dram_tensor("dq", [C, NQ], F32, kind="ExternalInput"),
        "r": nc.dram_tensor("r", [C, HW], F32, kind="ExternalInput"),
        "wqt4": nc.dram_tensor("wqt4", [C, P], F32, kind="ExternalInput"),
        "wkt4": nc.dram_tensor("wkt4", [C, P], F32, kind="ExternalInput"),
        "wvt": nc.dram_tensor("wvt", [C, C], F32, kind="ExternalInput"),
        "bq4": nc.dram_tensor("bq4", [P, 1], F32, kind="ExternalInput"),
        "bk4": nc.dram_tensor("bk4", [P, 1], F32, kind="ExternalInput"),
        "bv2": nc.dram_tensor("bv2", [C, 1], F32, kind="ExternalInput"),
        "out": nc.dram_tensor("out", [C, NQ], F32, kind="ExternalOutput"),
    }
    with tile.TileContext(nc) as tc:
        _emit(tc, io)
    nc.compile()
    _BUILT = nc
    return nc


def _in_maps(rgb, depth, Wq, bq, Wk, bk, Wv, bv):
    f = np.float32
    d_all = np.ascontiguousarray(depth.reshape(B, C, HW), dtype=f)
    r_all = np.ascontiguousarray(rgb.reshape(B, C, HW), dtype=f)
    wqt4 = np.ascontiguousarray(np.tile(np.asarray(Wq, f).T, (1, 4)))
    wkt4 = np.ascontiguousarray(np.tile(np.asarray(Wk, f).T, (1, 4)))
    wvt = np.ascontiguousarray(np.asarray(Wv, f).T)
    bq4 = np.ascontiguousarray(np.tile(np.asarray(bq, f), 4).reshape(P, 1))
    bk4 = np.ascontiguousarray(np.tile(np.asarray(bk, f), 4).reshape(P, 1))
    bv2 = np.ascontiguousarray(np.asarray(bv, f).reshape(C, 1))
    maps = []
    for core in range(8):
        b, half = core // 2, core % 2
        maps.append({
            "d": d_all[b],
            "dq": np.ascontiguousarray(d_all[b][:, half * NQ:(half + 1) * NQ]),
            "r": r_all[b],
            "wqt4": wqt4, "wkt4": wkt4, "wvt": wvt,
            "bq4": bq4, "bk4": bk4, "bv2": bv2,
        })
    return maps


def kernel(rgb, depth, Wq, bq, Wk, bk, Wv, bv, **run_kwargs):
    nc = _build()
    maps = _in_maps(rgb, depth, Wq, bq, Wk, bk, Wv, bv)
    res = run_bass_kernel_spmd(nc, maps, core_ids=list(range(8)), **run_kwargs)
    results = res.results if hasattr(res, "results") else res
    out = np.empty((B, C, HW), dtype=np.float32)
    for core in range(8):
        b, half = core // 2, core % 2
        out[b][:, half * NQ:(half + 1) * NQ] = results[core]["out"]
    kernel.last_results = res
    return out.reshape(B, C, H, W)



# revision 15
# speedup vs baseline: 1.3149x; 1.3149x over previous
"""CrossAttention (DFFNet) Trainium2 Bass kernel.

Shapes (hardcoded): rgb/depth [4, 256, 64, 64] f32; Wq/Wk [32, 256]; Wv [256, 256].

    q = Wq @ d + bq          [B, 32, 4096]
    k = Wk @ d + bk          [B, 32, 4096]
    v = Wv @ r + bv          [B, 256, 4096]
    scores = q^T k           [B, 4096, 4096], softmax over keys (last dim)
    feat = v @ mask^T        [B, 256, 4096]

Sharding: 8 cores = 4 batches x 2 query-halves (2048 queries each). Each core
gets full depth/rgb for its batch (keys/values span all 4096 tokens); the
query projection reads the core's half directly out of the depth tiles.

Device layout: scores are computed TRANSPOSED, st[m, n] (keys m on partitions,
queries n free) so the feat matmul needs no transposes:
  - v^T[m, c] is produced directly by  r-slice^T @ Wv^T; the value bias bv is
    folded into v^T (softmax rows sum to 1, so bias passes through feat).
  - feat[c, n] = sum_m v^T[m, c] * exp(st[m, n]) / S[n]
  - softmax denominator S[n]: DVE accumulates acc[:, n] += exp tiles
    (partition-partial sums); one tiny f32r ones-matmul per query tile
    finishes the partition reduction. This keeps the big reduction OFF the
    tensor engine (the baseline burned ~40us of PE time on ones-matmuls).
  - no max-subtraction: |scores| < ~6 here, exp is well-conditioned.

The K=32 score matmuls are row-packed in pairs (tile_position=(32j, 0)): k and
q are kept in 4x-replicated layouts [128, *], which fall out of the projection
matmuls for free by tiling the tiny weight matrices host-side.

Inputs are pre-cast to bf16 on the host (halves DMA bytes, no on-chip casts).
Main loop is software-pipelined one group deep: scores(g+1) issues before
feat(g) so the exp activation latency hides behind tensor-engine work.
"""

import numpy as np
import ml_dtypes

import concourse.bacc as bacc
import concourse.bass as bass
import concourse.mybir as mybir
import concourse.tile as tile
from concourse.bass_utils import run_bass_kernel_spmd

B, C, H, W = 4, 256, 64, 64
HW = H * W            # 4096
CQK = 32
P = 128
NQ = HW // 2          # 2048 queries per core
NT = 512              # query tile
N_NT = NQ // NT       # 4
MT = HW // P          # 32 key tiles
KC = C // P           # 2 contraction tiles for the projections
NG = MT // 2          # 16 score groups (2 key-tiles each) per query tile

F32 = mybir.dt.float32
F32R = mybir.dt.float32r
BF16 = mybir.dt.bfloat16
AF = mybir.ActivationFunctionType
OP = mybir.AluOpType
BF16_NP = ml_dtypes.bfloat16


def _emit(tc, io):
    nc = tc.nc
    d = io["d"].ap()          # [256, 4096] bf16 depth (keys + queries source)
    r = io["r"].ap()          # [256, 4096] bf16 rgb (values source)
    wqt4 = io["wqt4"].ap()    # [256, 128] bf16 = tile(Wq.T, (1,4))
    wkt4 = io["wkt4"].ap()    # [256, 128] bf16
    wvt = io["wvt"].ap()      # [256, 256] bf16 = Wv.T
    bq4 = io["bq4"].ap()      # [128, 1] f32 = tile(bq, 4)
    bk4 = io["bk4"].ap()      # [128, 1] f32
    bvr = io["bvr"].ap()      # [1, 256] f32 = bv row
    out = io["out"].ap()      # [256, 2048] f32
    # The host rotates the key axis of d and r per core so this core's
    # 2048 queries always sit at columns 0:2048 (softmax + value sum are
    # permutation-invariant over keys) -> one program for all 8 cores.

    from contextlib import ExitStack

    with ExitStack() as ctx:
        pw = ctx.enter_context(tc.tile_pool(name="weights", bufs=1))
        pin = ctx.enter_context(tc.tile_pool(name="inputs", bufs=1))
        pqk = ctx.enter_context(tc.tile_pool(name="qk", bufs=1))
        pvt = ctx.enter_context(tc.tile_pool(name="vt", bufs=1))
        pse = ctx.enter_context(tc.tile_pool(name="stexp", bufs=3))
        pacc = ctx.enter_context(tc.tile_pool(name="accp", bufs=2))
        psmall = ctx.enter_context(tc.tile_pool(name="small", bufs=2))
        pout = ctx.enter_context(tc.tile_pool(name="outsb", bufs=4))
        # PSUM: 8 banks of [128, 512] f32 total.
        ps_st = ctx.enter_context(       # scores [128,1024] x2 = 4 banks
            tc.tile_pool(name="ps_st", bufs=2, space=bass.MemorySpace.PSUM))
        ps_feat = ctx.enter_context(     # fc0+fc1 [128,512] x2 = 2 banks
            tc.tile_pool(name="ps_feat", bufs=2, space=bass.MemorySpace.PSUM))
        ps_aux = ctx.enter_context(      # vp / sm / bc rotate = 2 banks
            tc.tile_pool(name="ps_aux", bufs=2, space=bass.MemorySpace.PSUM))

        # ---- weights / constants (DMA straight to SBUF, already bf16) ----
        wq_t, wk_t, wv_t = [], [], []
        for kc in range(KC):
            t = pw.tile([P, P], BF16, tag=f"wq{kc}")
            nc.sync.dma_start(t[:], wqt4[kc * P:(kc + 1) * P, :])
            wq_t.append(t)
            t = pw.tile([P, P], BF16, tag=f"wk{kc}")
            nc.sync.dma_start(t[:], wkt4[kc * P:(kc + 1) * P, :])
            wk_t.append(t)
            t = pw.tile([P, C], BF16, tag=f"wv{kc}")
            nc.sync.dma_start(t[:], wvt[kc * P:(kc + 1) * P, :])
            wv_t.append(t)
        bq_sb = pw.tile([P, 1], F32, tag="bq")
        nc.sync.dma_start(bq_sb[:], bq4[:])
        bk_sb = pw.tile([P, 1], F32, tag="bk")
        nc.sync.dma_start(bk_sb[:], bk4[:])
        bv_row = pw.tile([1, C], F32, tag="bvr")
        nc.sync.dma_start(bv_row[:], bvr[:])
        ones_row = pw.tile([1, P], BF16, tag="ones_row")
        nc.vector.memset(ones_row[:], 1.0)
        ones_col = pw.tile([P, 1], BF16, tag="ones_col")
        nc.vector.memset(ones_col[:], 1.0)
        ones_row_f = pw.tile([1, P], F32, tag="ones_row_f")
        nc.vector.memset(ones_row_f[:], 1.0)

        # ---- inputs: d first (k/q proj), then r (v proj) ------------------
        def _load(dram_ap, kc, ch, pref):
            t = pin.tile([P, 1024], BF16, tag=f"{pref}{kc}_{ch}",
                         name=f"{pref}{kc}_{ch}")
            nc.sync.dma_start(
                t[:], dram_ap[kc * P:(kc + 1) * P, ch * 1024:(ch + 1) * 1024])
            return t

        # DMA order = consumption order: d chunks 0,1 feed the q projection
        # (queries live at columns 0:2048 after the host-side key rotation),
        # all of d feeds k, then r feeds v.
        d_sb = [[None] * 4 for _ in range(KC)]
        for ch in range(4):
            for kc in range(KC):
                d_sb[kc][ch] = _load(d, kc, ch, "d")
        r_sb = [[_load(r, kc, ch, "r") for ch in range(4)] for kc in range(KC)]

        # bv broadcast to 128 partitions: bvb[p, c] = bv[c] (one K=1 matmul;
        # plain fp32 — one-time, and the DMA'd bias isn't f32r-rounded).
        bvp = ps_aux.tile([P, C], F32, tag="aux", name="bvp")
        nc.tensor.matmul(bvp[:], lhsT=ones_row_f[:], rhs=bv_row[:],
                         start=True, stop=True)
        bvb = pw.tile([P, C], F32, tag="bvb")
        nc.vector.tensor_copy(bvb[:], bvp[:])

        # ---- q projection (4x-replicated): q4[32j+o, n] = q[o, n] ---------
        q4 = pqk.tile([P, NQ], BF16, tag="q4")
        for qh in range(2):
            qp = ps_st.tile([P, 1024], F32, tag="stp", name=f"qp{qh}")
            for sub in range(2):
                n0 = sub * NT
                for kc in range(KC):
                    nc.tensor.matmul(
                        qp[:, n0:n0 + NT],
                        lhsT=wq_t[kc][:],
                        rhs=d_sb[kc][qh][:, n0:n0 + NT],
                        start=(kc == 0),
                        stop=(kc == KC - 1),
                    )
            nc.vector.tensor_scalar(
                q4[:, qh * 1024:(qh + 1) * 1024], qp[:], bq_sb[:], None, OP.add
            )

        # ---- k projection (4x-replicated over all 4096 keys) --------------
        k4 = pqk.tile([P, HW], BF16, tag="k4")
        for qtr in range(4):
            kp = ps_st.tile([P, 1024], F32, tag="stp", name=f"kp{qtr}")
            for sub in range(2):
                n0 = sub * NT
                for kc in range(KC):
                    nc.tensor.matmul(
                        kp[:, n0:n0 + NT],
                        lhsT=wk_t[kc][:],
                        rhs=d_sb[kc][qtr][:, n0:n0 + NT],
                        start=(kc == 0),
                        stop=(kc == KC - 1),
                    )
            nc.vector.tensor_scalar(
                k4[:, qtr * 1024:(qtr + 1) * 1024], kp[:], bk_sb[:], None, OP.add
            )

        # ---- v^T projection: vt[mt][p, c] = v[c, mt*128 + p] + bv[c] ------
        vt_t = []
        for mt in range(MT):
            vp = ps_aux.tile([P, C], F32, tag="aux", name=f"vp{mt}")
            for kc in range(KC):
                nc.tensor.matmul(
                    vp[:],
                    lhsT=r_sb[kc][mt // 8][:, (mt % 8) * P:(mt % 8 + 1) * P],
                    rhs=wv_t[kc][:],
                    start=(kc == 0),
                    stop=(kc == KC - 1),
                )
            t = pvt.tile([P, C], BF16, tag=f"vt{mt}")
            nc.vector.tensor_tensor(t[:], vp[:], bvb[:], OP.add)
            vt_t.append(t)

        # ---- main attention loop (software-pipelined one group deep) ------
        # group i = (nt, g): scores+exp+acc for i issue before feat for i-1.
        # acc is double-buffered per query tile: tail(nt) reads acc[nt] while
        # front(nt+1) already writes acc[nt+1] (DVE runs in program order).
        acc_t = [None] * N_NT
        accf = pacc.tile([P, NT], BF16, tag="accf")
        se_t = [None] * (N_NT * NG)
        fc = None

        def emit_front(i):
            nt, g = divmod(i, NG)
            stp = ps_st.tile([P, 1024], F32, tag="stp", name=f"stp{i}")
            n0 = nt * NT
            for j in range(2):
                mt = 2 * g + j
                nc.tensor.matmul(
                    stp[:, j * NT:(j + 1) * NT],
                    lhsT=k4[32 * j:32 * j + 32, mt * P:(mt + 1) * P],
                    rhs=q4[32 * j:32 * j + 32, n0:n0 + NT],
                    start=True,
                    stop=True,
                    tile_position=(32 * j, 0),
                )
            se = pse.tile([P, 1024], BF16, tag="se", name=f"se{i}")
            nc.scalar.activation(se[:], stp[:], AF.Exp)
            se_t[i] = se
            if g == 0:
                acc_t[nt] = pacc.tile([P, 1024], F32, tag="acc",
                                      name=f"acc{nt}")
                nc.vector.tensor_copy(acc_t[nt][:], se[:])
            else:
                nc.vector.tensor_tensor(acc_t[nt][:], acc_t[nt][:], se[:],
                                        OP.add)

        def emit_feat(i):
            nt, g = divmod(i, NG)
            se = se_t[i]
            for j in range(2):
                mt = 2 * g + j
                sej = se[:, j * NT:(j + 1) * NT]
                first = mt == 0
                last = mt == MT - 1
                nc.tensor.matmul(
                    fc[0][:], lhsT=vt_t[mt][:, 0:P], rhs=sej,
                    start=first, stop=last,
                )
                nc.tensor.matmul(
                    fc[1][:], lhsT=vt_t[mt][:, P:C], rhs=sej,
                    start=first, stop=last,
                )
            se_t[i] = None

        def emit_tail(nt):
            # partition-reduce acc -> S[n], then 1/S broadcast + normalize.
            n0 = nt * NT
            acc = acc_t[nt]
            nc.vector.tensor_tensor(
                accf[:], acc[:, 0:NT], acc[:, NT:1024], OP.add)
            sm = ps_aux.tile([1, NT], F32, tag="aux", name=f"sm{nt}")
            nc.tensor.matmul(
                sm[:], lhsT=ones_col[:], rhs=accf[:], start=True, stop=True)
            rc = psmall.tile([1, NT], F32, tag="recip")
            nc.vector.reciprocal_approx_fast(out=rc[:], in_=sm[:])
            rc_h = psmall.tile([1, NT], BF16, tag="recip_h")
            nc.vector.tensor_copy(rc_h[:], rc[:])
            bc = ps_aux.tile([P, NT], F32, tag="aux", name=f"bc{nt}")
            nc.tensor.matmul(
                bc[:], lhsT=ones_row[:], rhs=rc_h[:], start=True, stop=True)
            bc_sb = pout.tile([P, NT], F32, tag="bc_sb")
            nc.vector.tensor_copy(bc_sb[:], bc[:])
            for c in range(2):
                ot = pout.tile([P, NT], F32, tag="ot")
                nc.vector.tensor_tensor(ot[:], fc[c][:], bc_sb[:], OP.mult)
                nc.sync.dma_start(out[c * P:(c + 1) * P, n0:n0 + NT], ot[:])

        NTOT = N_NT * NG
        for i in range(NTOT + 1):
            if i < NTOT:
                if i % NG == 0:
                    fc_new = [
                        ps_feat.tile([P, NT], F32, tag="feat",
                                     name=f"fc{i // NG}_{c}")
                        for c in range(2)
                    ]
                emit_front(i)
            if i >= 1:
                nt_p, g_p = divmod(i - 1, NG)
                if g_p == 0:
                    fc = fc_new
                emit_feat(i - 1)
                if g_p == NG - 1:
                    emit_tail(nt_p)


_BUILT = None


def _build():
    global _BUILT
    if _BUILT is not None:
        return _BUILT
    nc = bacc.Bacc("TRN2", target_bir_lowering=False, debug=False)
    io = {
        "d": nc.dram_tensor("d", [C, HW], BF16, kind="ExternalInput"),
        "r": nc.dram_tensor("r", [C, HW], BF16, kind="ExternalInput"),
        "wqt4": nc.dram_tensor("wqt4", [C, P], BF16, kind="ExternalInput"),
        "wkt4": nc.dram_tensor("wkt4", [C, P], BF16, kind="ExternalInput"),
        "wvt": nc.dram_tensor("wvt", [C, C], BF16, kind="ExternalInput"),
        "bq4": nc.dram_tensor("bq4", [P, 1], F32, kind="ExternalInput"),
        "bk4": nc.dram_tensor("bk4", [P, 1], F32, kind="ExternalInput"),
        "bvr": nc.dram_tensor("bvr", [1, C], F32, kind="ExternalInput"),
        "out": nc.dram_tensor("out", [C, NQ], F32, kind="ExternalOutput"),
    }
    with tile.TileContext(nc) as tc:
        _emit(tc, io)
    nc.compile()
    _BUILT = nc
    return nc


def _in_maps(rgb, depth, Wq, bq, Wk, bk, Wv, bv):
    f = np.float32
    d_all = np.asarray(depth, f).reshape(B, C, HW).astype(BF16_NP)
    r_all = np.asarray(rgb, f).reshape(B, C, HW).astype(BF16_NP)
    wqt4 = np.ascontiguousarray(
        np.tile(np.asarray(Wq, f).T, (1, 4)).astype(BF16_NP))
    wkt4 = np.ascontiguousarray(
        np.tile(np.asarray(Wk, f).T, (1, 4)).astype(BF16_NP))
    wvt = np.ascontiguousarray(np.asarray(Wv, f).T.astype(BF16_NP))
    bq4 = np.ascontiguousarray(np.tile(np.asarray(bq, f), 4).reshape(P, 1))
    bk4 = np.ascontiguousarray(np.tile(np.asarray(bk, f), 4).reshape(P, 1))
    bvr = np.ascontiguousarray(np.asarray(bv, f).reshape(1, C))
    maps = []
    for core in range(8):
        b, half = core // 2, core % 2
        # Rotate the key axis so this core's query half sits at cols 0:2048;
        # softmax + the value reduction are permutation-invariant over keys
        # as long as d and r use the same rotation.
        rot = np.r_[half * NQ:(half * NQ + HW)] % HW
        maps.append({
            "d": np.ascontiguousarray(d_all[b][:, rot]),
            "r": np.ascontiguousarray(r_all[b][:, rot]),
            "wqt4": wqt4, "wkt4": wkt4, "wvt": wvt,
            "bq4": bq4, "bk4": bk4, "bvr": bvr,
        })
    return maps


def kernel(rgb, depth, Wq, bq, Wk, bk, Wv, bv, **run_kwargs):
    nc = _build()
    maps = _in_maps(rgb, depth, Wq, bq, Wk, bk, Wv, bv)
    res = run_bass_kernel_spmd(nc, maps, core_ids=list(range(8)), **run_kwargs)
    results = res.results if hasattr(res, "results") else res
    out = np.empty((B, C, HW), dtype=np.float32)
    for core in range(8):
        b, half = core // 2, core % 2
        out[b][:, half * NQ:(half + 1) * NQ] = results[core]["out"]
    kernel.last_results = res
    return out.reshape(B, C, H, W)


# revision 17
# speedup vs baseline: 1.8176x; 1.3824x over previous
"""CrossAttention (DFFNet) Trainium2 Bass kernel.

Shapes (hardcoded): rgb/depth [4, 256, 64, 64] f32; Wq/Wk [32, 256]; Wv [256, 256].

    q = Wq @ d + bq          [B, 32, 4096]
    k = Wk @ d + bk          [B, 32, 4096]
    v = Wv @ r + bv          [B, 256, 4096]
    scores = q^T k           [B, 4096, 4096], softmax over keys (last dim)
    feat = v @ mask^T        [B, 256, 4096]

Sharding: 8 cores = 4 batches x 2 query-halves (2048 queries each). The host
rotates the key axis of d and r per core so the core's queries sit at columns
0:2048 (softmax + value reduction are permutation-invariant over keys), so a
single program serves all 8 cores.

Device layout: scores are computed TRANSPOSED, st[m, n] (keys m on partitions,
queries n free) so the feat matmul needs no transposes:
  - v^T[m, c] = r-slice^T @ Wv^T + bv (bias folded in; softmax rows sum to 1
    so the value bias passes straight through feat).
  - feat[c, n] = sum_m v^T[m, c] * exp(st[m, n]) / S[n]
  - S[n]: DVE accumulates acc[:, n] += exp tiles (fp16, fast DVE mode);
    one tiny bf16 ones-matmul per query tile finishes the partition
    reduction. Keeps the big reduction OFF the tensor engine.
  - no max-subtraction: |scores| < ~6, exp is well-conditioned.

Engine budget per 2-key-tile group (steady state): PE = score pair (row-packed
K=32 at tile_position 32j) + 4 feat matmuls ~1.2us; ACT = one [128,1024] exp
~1.34us; DVE = one fp16 acc add. The loop is software-pipelined one group
deep (scores(i) issues before feat(i-1)); at query-tile boundaries fc is
copied PSUM->SBUF immediately so the next tile's feat accumulation starts
without waiting for the softmax-normalize chain, and the sums/broadcast
matmuls are staggered across the next two slots.

Inputs are pre-cast to bf16 on the host and DMA'd in a few large transfers
(the DMA queue costs ~0.7us per descriptor regardless of size).
"""

import numpy as np
import ml_dtypes

import concourse.bacc as bacc
import concourse.bass as bass
import concourse.mybir as mybir
import concourse.tile as tile
from concourse.bass_utils import run_bass_kernel_spmd

B, C, H, W = 4, 256, 64, 64
HW = H * W            # 4096
CQK = 32
P = 128
NQ = HW // 2          # 2048 queries per core
NT = 512              # query tile
N_NT = NQ // NT       # 4
MT = HW // P          # 32 key tiles
KC = C // P           # 2 contraction tiles for the projections
NG = MT // 2          # 16 score groups (2 key-tiles each) per query tile
NTOT = N_NT * NG      # 64 groups

F32 = mybir.dt.float32
F16 = mybir.dt.float16
BF16 = mybir.dt.bfloat16
AF = mybir.ActivationFunctionType
OP = mybir.AluOpType
BF16_NP = ml_dtypes.bfloat16


def _emit(tc, io):
    nc = tc.nc
    d = io["d"].ap()          # [256, 4096] bf16 depth (keys + queries source)
    r = io["r"].ap()          # [256, 4096] bf16 rgb (values source)
    wcat = io["wcat"].ap()    # [256, 512] bf16 = [tile(Wq.T,(1,4)) | tile(Wk.T,(1,4)) | Wv.T]
    bqk = io["bqk"].ap()      # [128, 2] f32 = [tile(bq,4) | tile(bk,4)]
    bvr = io["bvr"].ap()      # [1, 256] f32 = bv row
    out = io["out"].ap()      # [256, 2048] f32

    from contextlib import ExitStack

    with ExitStack() as ctx:
        pw = ctx.enter_context(tc.tile_pool(name="weights", bufs=1))
        pin = ctx.enter_context(tc.tile_pool(name="inputs", bufs=1))
        pqk = ctx.enter_context(tc.tile_pool(name="qk", bufs=1))
        pvt = ctx.enter_context(tc.tile_pool(name="vt", bufs=1))
        pse = ctx.enter_context(tc.tile_pool(name="stexp", bufs=3))
        pacc = ctx.enter_context(tc.tile_pool(name="accp", bufs=2))
        pfcs = ctx.enter_context(tc.tile_pool(name="fcsb", bufs=4))
        psmall = ctx.enter_context(tc.tile_pool(name="small", bufs=2))
        pout = ctx.enter_context(tc.tile_pool(name="outsb", bufs=4))
        # PSUM: 8 banks of [128, 512] f32 total.
        ps_st = ctx.enter_context(       # scores [128,1024] x2 = 4 banks
            tc.tile_pool(name="ps_st", bufs=2, space=bass.MemorySpace.PSUM))
        ps_feat = ctx.enter_context(     # fc0+fc1 [128,512] x2 = 2 banks
            tc.tile_pool(name="ps_feat", bufs=2, space=bass.MemorySpace.PSUM))
        ps_aux = ctx.enter_context(      # vp / sm / bc rotate = 2 banks
            tc.tile_pool(name="ps_aux", bufs=2, space=bass.MemorySpace.PSUM))

        # ---- weights (one coalesced DMA per kc row-block) -----------------
        wc_sb = []
        for kc in range(KC):
            t = pw.tile([P, 512], BF16, tag=f"wc{kc}")
            nc.sync.dma_start(t[:], wcat[kc * P:(kc + 1) * P, :])
            wc_sb.append(t)
        wq_t = [wc_sb[kc][:, 0:P] for kc in range(KC)]
        wk_t = [wc_sb[kc][:, P:2 * P] for kc in range(KC)]
        wv_t = [wc_sb[kc][:, 2 * P:2 * P + C] for kc in range(KC)]
        bqk_sb = pw.tile([P, 2], F32, tag="bqk")
        nc.sync.dma_start(bqk_sb[:], bqk[:])
        bq_sb = bqk_sb[:, 0:1]
        bk_sb = bqk_sb[:, 1:2]
        bv_row = pw.tile([1, C], F32, tag="bvr")
        nc.sync.dma_start(bv_row[:], bvr[:])
        ones_row = pw.tile([1, P], BF16, tag="ones_row")
        nc.vector.memset(ones_row[:], 1.0)
        ones_col = pw.tile([P, 1], F16, tag="ones_col")
        nc.vector.memset(ones_col[:], 1.0)
        ones_row_f = pw.tile([1, P], F32, tag="ones_row_f")
        nc.vector.memset(ones_row_f[:], 1.0)

        # ---- inputs: 2 half-row DMAs per [128, 4096] block, ordered so the
        # q/k projections (cols 0:2048 of both kc blocks) unblock first.
        d_sb = [pin.tile([P, HW], BF16, tag=f"d{kc}", name=f"d{kc}")
                for kc in range(KC)]
        r_sb = [pin.tile([P, HW], BF16, tag=f"r{kc}", name=f"r{kc}")
                for kc in range(KC)]
        for src, dst in ((d, d_sb), (r, r_sb)):
            for h0 in (0, NQ):
                for kc in range(KC):
                    nc.sync.dma_start(
                        dst[kc][:, h0:h0 + NQ],
                        src[kc * P:(kc + 1) * P, h0:h0 + NQ])

        # bv broadcast to 128 partitions: bvb[p, c] = bv[c] (one K=1 matmul).
        bvp = ps_aux.tile([P, C], F32, tag="aux", name="bvp")
        nc.tensor.matmul(bvp[:], lhsT=ones_row_f[:], rhs=bv_row[:],
                         start=True, stop=True)
        bvb = pw.tile([P, C], F32, tag="bvb")
        nc.vector.tensor_copy(bvb[:], bvp[:])

        # ---- q projection (4x-replicated): q4[32j+o, n] = q[o, n] ---------
        # bias adds ride the scalar engine (idle during projections).
        q4 = pqk.tile([P, NQ], BF16, tag="q4")
        for qh in range(2):
            qp = ps_st.tile([P, 1024], F32, tag="stp", name=f"qp{qh}")
            for sub in range(2):
                n0 = sub * NT
                g0 = qh * 1024 + n0
                for kc in range(KC):
                    nc.tensor.matmul(
                        qp[:, n0:n0 + NT],
                        lhsT=wq_t[kc],
                        rhs=d_sb[kc][:, g0:g0 + NT],
                        start=(kc == 0),
                        stop=(kc == KC - 1),
                    )
            nc.scalar.activation(
                q4[:, qh * 1024:(qh + 1) * 1024], qp[:], AF.Identity,
                bias=bq_sb)

        # ---- k projection (4x-replicated over all 4096 keys) --------------
        k4 = pqk.tile([P, HW], BF16, tag="k4")
        for qtr in range(4):
            kp = ps_st.tile([P, 1024], F32, tag="stp", name=f"kp{qtr}")
            for sub in range(2):
                n0 = sub * NT
                g0 = qtr * 1024 + n0
                for kc in range(KC):
                    nc.tensor.matmul(
                        kp[:, n0:n0 + NT],
                        lhsT=wk_t[kc],
                        rhs=d_sb[kc][:, g0:g0 + NT],
                        start=(kc == 0),
                        stop=(kc == KC - 1),
                    )
            nc.scalar.activation(
                k4[:, qtr * 1024:(qtr + 1) * 1024], kp[:], AF.Identity,
                bias=bk_sb)

        # ---- v^T projection: vt[mt][p, c] = v[c, mt*128 + p] + bv[c] ------
        vt_t = []
        for mt in range(MT):
            vp = ps_aux.tile([P, C], F32, tag="aux", name=f"vp{mt}")
            for kc in range(KC):
                nc.tensor.matmul(
                    vp[:],
                    lhsT=r_sb[kc][:, mt * P:(mt + 1) * P],
                    rhs=wv_t[kc],
                    start=(kc == 0),
                    stop=(kc == KC - 1),
                )
            t = pvt.tile([P, C], BF16, tag=f"vt{mt}")
            nc.vector.tensor_tensor(t[:], vp[:], bvb[:], OP.add)
            vt_t.append(t)

        # ---- main attention loop ------------------------------------------
        acc_t = [None] * N_NT
        se_t = [None] * NTOT
        fc_t = [None] * N_NT     # PSUM accumulators (rotating 2 banks)
        fcs_t = [None] * N_NT    # SBUF copies
        sm_t = [None] * N_NT
        rc_t = [None] * N_NT
        bc_t = [None] * N_NT

        def emit_scores_exp(i):
            nt, g = divmod(i, NG)
            stp = ps_st.tile([P, 1024], F32, tag="stp", name=f"stp{i}")
            n0 = nt * NT
            for j in range(2):
                mt = 2 * g + j
                nc.tensor.matmul(
                    stp[:, j * NT:(j + 1) * NT],
                    lhsT=k4[32 * j:32 * j + 32, mt * P:(mt + 1) * P],
                    rhs=q4[32 * j:32 * j + 32, n0:n0 + NT],
                    start=True,
                    stop=True,
                    tile_position=(32 * j, 0),
                )
            se = pse.tile([P, 1024], BF16, tag="se", name=f"se{i}")
            nc.scalar.activation(se[:], stp[:], AF.Exp)
            se_t[i] = se

        def emit_acc(i):
            nt, g = divmod(i, NG)
            if g == 0:
                acc_t[nt] = pacc.tile([P, 1024], F16, tag="acc",
                                      name=f"acc{nt}")
                nc.vector.tensor_copy(acc_t[nt][:], se_t[i][:])
            else:
                nc.vector.tensor_tensor(acc_t[nt][:], acc_t[nt][:],
                                        se_t[i][:], OP.add)

        def emit_feat(i):
            nt, g = divmod(i, NG)
            if g == 0:
                fc_t[nt] = [
                    ps_feat.tile([P, NT], F32, tag="feat",
                                 name=f"fc{nt}_{c}") for c in range(2)]
            fc = fc_t[nt]
            se = se_t[i]
            for j in range(2):
                mt = 2 * g + j
                sej = se[:, j * NT:(j + 1) * NT]
                first = mt == 0
                last = mt == MT - 1
                nc.tensor.matmul(
                    fc[0][:], lhsT=vt_t[mt][:, 0:P], rhs=sej,
                    start=first, stop=last,
                )
                nc.tensor.matmul(
                    fc[1][:], lhsT=vt_t[mt][:, P:C], rhs=sej,
                    start=first, stop=last,
                )
            se_t[i] = None

        def emit_fc_free(nt):
            # Copy fc out of PSUM right away so the next tile's accumulation
            # can claim the banks without waiting for the normalize chain.
            fcs_t[nt] = []
            for c in range(2):
                t = pfcs.tile([P, NT], F32, tag="fcs", name=f"fcs{nt}_{c}")
                nc.vector.tensor_copy(t[:], fc_t[nt][c][:])
                fcs_t[nt].append(t)

        def emit_fold(nt):
            accf = pacc.tile([P, NT], F16, tag="accf", name=f"accf{nt}")
            acc = acc_t[nt]
            nc.vector.tensor_tensor(
                accf[:], acc[:, 0:NT], acc[:, NT:1024], OP.add)
            acc_t[nt] = accf  # reuse slot to pass to emit_sm

        def emit_sm(nt):
            sm = ps_aux.tile([1, NT], F32, tag="aux", name=f"sm{nt}")
            nc.tensor.matmul(sm[:], lhsT=ones_col[:], rhs=acc_t[nt][:],
                             start=True, stop=True)
            sm_t[nt] = sm

        def emit_recip(nt):
            rc = psmall.tile([1, NT], F32, tag="recip")
            nc.vector.reciprocal_approx_fast(out=rc[:], in_=sm_t[nt][:])
            rc_h = psmall.tile([1, NT], BF16, tag="recip_h")
            nc.vector.tensor_copy(rc_h[:], rc[:])
            rc_t[nt] = rc_h

        def emit_bc(nt):
            bc = ps_aux.tile([P, NT], F32, tag="aux", name=f"bc{nt}")
            nc.tensor.matmul(bc[:], lhsT=ones_row[:], rhs=rc_t[nt][:],
                             start=True, stop=True)
            bc_t[nt] = bc

        def emit_norm_out(nt):
            n0 = nt * NT
            bc_sb = pout.tile([P, NT], F32, tag="bc_sb")
            nc.vector.tensor_copy(bc_sb[:], bc_t[nt][:])
            for c in range(2):
                ot = pout.tile([P, NT], F32, tag="ot")
                nc.vector.tensor_tensor(ot[:], fcs_t[nt][c][:], bc_sb[:],
                                        OP.mult)
                nc.sync.dma_start(out[c * P:(c + 1) * P, n0:n0 + NT], ot[:])

        # Slot schedule: front work for group i, feat for i-1; tile tails are
        # staggered so no PE instruction waits on a long DVE chain.
        for i in range(NTOT + 3):
            if i < NTOT:
                emit_scores_exp(i)
            if 1 <= i <= NTOT:
                emit_feat(i - 1)
                if (i - 1) % NG == NG - 1:
                    emit_fc_free((i - 1) // NG)
                    emit_fold((i - 1) // NG)
            if i < NTOT:
                emit_acc(i)
            # tails for tile nt run at slots nt*NG+NG+1 (sm) / +2 (bc, norm)
            if i >= NG + 1 and (i - NG - 1) % NG == 0:
                nt = (i - NG - 1) // NG
                emit_sm(nt)
                emit_recip(nt)
            if i >= NG + 2 and (i - NG - 2) % NG == 0:
                nt = (i - NG - 2) // NG
                emit_bc(nt)
                emit_norm_out(nt)


_BUILT = None


def _build():
    global _BUILT
    if _BUILT is not None:
        return _BUILT
    nc = bacc.Bacc("TRN2", target_bir_lowering=False, debug=False)
    io = {
        "d": nc.dram_tensor("d", [C, HW], BF16, kind="ExternalInput"),
        "r": nc.dram_tensor("r", [C, HW], BF16, kind="ExternalInput"),
        "wcat": nc.dram_tensor("wcat", [C, 512], BF16, kind="ExternalInput"),
        "bqk": nc.dram_tensor("bqk", [P, 2], F32, kind="ExternalInput"),
        "bvr": nc.dram_tensor("bvr", [1, C], F32, kind="ExternalInput"),
        "out": nc.dram_tensor("out", [C, NQ], F32, kind="ExternalOutput"),
    }
    with tile.TileContext(nc) as tc:
        _emit(tc, io)
    nc.compile()
    _BUILT = nc
    return nc


def _in_maps(rgb, depth, Wq, bq, Wk, bk, Wv, bv):
    f = np.float32
    d_all = np.asarray(depth, f).reshape(B, C, HW).astype(BF16_NP)
    r_all = np.asarray(rgb, f).reshape(B, C, HW).astype(BF16_NP)
    wqt4 = np.tile(np.asarray(Wq, f).T, (1, 4))
    wkt4 = np.tile(np.asarray(Wk, f).T, (1, 4))
    wvt = np.asarray(Wv, f).T
    wcat = np.ascontiguousarray(
        np.concatenate([wqt4, wkt4, wvt], axis=1).astype(BF16_NP))
    bqk = np.ascontiguousarray(
        np.stack([np.tile(np.asarray(bq, f), 4),
                  np.tile(np.asarray(bk, f), 4)], axis=1))
    bvr = np.ascontiguousarray(np.asarray(bv, f).reshape(1, C))
    maps = []
    for core in range(8):
        b, half = core // 2, core % 2
        # Rotate the key axis so this core's query half sits at cols 0:2048;
        # softmax + the value reduction are permutation-invariant over keys
        # as long as d and r use the same rotation.
        rot = np.r_[half * NQ:(half * NQ + HW)] % HW
        maps.append({
            "d": np.ascontiguousarray(d_all[b][:, rot]),
            "r": np.ascontiguousarray(r_all[b][:, rot]),
            "wcat": wcat, "bqk": bqk, "bvr": bvr,
        })
    return maps


def kernel(rgb, depth, Wq, bq, Wk, bk, Wv, bv, **run_kwargs):
    nc = _build()
    maps = _in_maps(rgb, depth, Wq, bq, Wk, bk, Wv, bv)
    res = run_bass_kernel_spmd(nc, maps, core_ids=list(range(8)), **run_kwargs)
    results = res.results if hasattr(res, "results") else res
    out = np.empty((B, C, HW), dtype=np.float32)
    for core in range(8):
        b, half = core // 2, core % 2
        out[b][:, half * NQ:(half + 1) * NQ] = results[core]["out"]
    kernel.last_results = res
    return out.reshape(B, C, H, W)


# revision 20
# speedup vs baseline: 1.8381x; 1.0113x over previous
"""CrossAttention (DFFNet) Trainium2 Bass kernel.

Shapes (hardcoded): rgb/depth [4, 256, 64, 64] f32; Wq/Wk [32, 256]; Wv [256, 256].

    q = Wq @ d + bq          [B, 32, 4096]
    k = Wk @ d + bk          [B, 32, 4096]
    v = Wv @ r + bv          [B, 256, 4096]
    scores = q^T k           [B, 4096, 4096], softmax over keys (last dim)
    feat = v @ mask^T        [B, 256, 4096]

Sharding: 8 cores = 4 batches x 2 query-halves (2048 queries each). The host
rotates the key axis of d and r per core so the core's queries sit at columns
0:2048 (softmax + value reduction are permutation-invariant over keys), so a
single program serves all 8 cores.

Device layout: scores are computed TRANSPOSED, st[m, n] (keys m on partitions,
queries n free) so the feat matmul needs no transposes:
  - v^T[m, c] = r-slice^T @ Wv^T + bv (bias folded in; softmax rows sum to 1
    so the value bias passes straight through feat).
  - feat[c, n] = sum_m v^T[m, c] * exp(st[m, n]) / S[n]
  - S[n]: DVE accumulates acc[:, n] += exp tiles (fp16, fast DVE mode);
    one tiny bf16 ones-matmul per query tile finishes the partition
    reduction. Keeps the big reduction OFF the tensor engine.
  - no max-subtraction: |scores| < ~6, exp is well-conditioned.

Engine budget per 2-key-tile group (steady state): PE = score pair (row-packed
K=32 at tile_position 32j) + 4 feat matmuls ~1.2us; ACT = one [128,1024] exp
~1.34us; DVE = one fp16 acc add. The loop is software-pipelined one group
deep (scores(i) issues before feat(i-1)); at query-tile boundaries fc is
copied PSUM->SBUF immediately so the next tile's feat accumulation starts
without waiting for the softmax-normalize chain, and the sums/broadcast
matmuls are staggered across the next two slots.

Inputs are pre-cast to bf16 on the host and DMA'd in a few large transfers
(the DMA queue costs ~0.7us per descriptor regardless of size).
"""

import numpy as np
import ml_dtypes

import concourse.bacc as bacc
import concourse.bass as bass
import concourse.mybir as mybir
import concourse.tile as tile
from concourse.bass_utils import run_bass_kernel_spmd

B, C, H, W = 4, 256, 64, 64
HW = H * W            # 4096
CQK = 32
P = 128
NQ = HW // 2          # 2048 queries per core
NT = 512              # query tile
N_NT = NQ // NT       # 4
MT = HW // P          # 32 key tiles
KC = C // P           # 2 contraction tiles for the projections
NG = MT // 2          # 16 score groups (2 key-tiles each) per query tile
NTOT = N_NT * NG      # 64 groups

F32 = mybir.dt.float32
F16 = mybir.dt.float16
BF16 = mybir.dt.bfloat16
AF = mybir.ActivationFunctionType
OP = mybir.AluOpType
BF16_NP = ml_dtypes.bfloat16


def _emit(tc, io):
    nc = tc.nc
    d = io["d"].ap()          # [256, 4096] bf16 depth (keys + queries source)
    r = io["r"].ap()          # [256, 4096] bf16 rgb (values source)
    wcat = io["wcat"].ap()    # [256, 512] bf16 = [tile(Wq.T,(1,4)) | tile(Wk.T,(1,4)) | Wv.T]
    bqk = io["bqk"].ap()      # [128, 2] f32 = [tile(bq,4) | tile(bk,4)]
    bvr = io["bvr"].ap()      # [1, 256] f32 = bv row
    out = io["out"].ap()      # [256, 2048] f32

    from contextlib import ExitStack

    with ExitStack() as ctx:
        pw = ctx.enter_context(tc.tile_pool(name="weights", bufs=1))
        pin = ctx.enter_context(tc.tile_pool(name="inputs", bufs=1))
        pqk = ctx.enter_context(tc.tile_pool(name="qk", bufs=1))
        pvt = ctx.enter_context(tc.tile_pool(name="vt", bufs=1))
        pse = ctx.enter_context(tc.tile_pool(name="stexp", bufs=4))
        pacc = ctx.enter_context(tc.tile_pool(name="accp", bufs=2))
        pfcs = ctx.enter_context(tc.tile_pool(name="fcsb", bufs=4))
        psmall = ctx.enter_context(tc.tile_pool(name="small", bufs=2))
        pout = ctx.enter_context(tc.tile_pool(name="outsb", bufs=4))
        # PSUM: 8 banks of [128, 512] f32 total.
        ps_st = ctx.enter_context(       # scores [128,1024] x2 = 4 banks
            tc.tile_pool(name="ps_st", bufs=2, space=bass.MemorySpace.PSUM))
        ps_feat = ctx.enter_context(     # fc0+fc1 [128,512] x2 = 2 banks
            tc.tile_pool(name="ps_feat", bufs=2, space=bass.MemorySpace.PSUM))
        ps_aux = ctx.enter_context(      # vp / sm / bc rotate = 2 banks
            tc.tile_pool(name="ps_aux", bufs=2, space=bass.MemorySpace.PSUM))

        # ---- weights (one coalesced DMA per kc row-block) -----------------
        wc_sb = []
        for kc in range(KC):
            t = pw.tile([P, 512], BF16, tag=f"wc{kc}")
            nc.sync.dma_start(t[:], wcat[kc * P:(kc + 1) * P, :])
            wc_sb.append(t)
        wq_t = [wc_sb[kc][:, 0:P] for kc in range(KC)]
        wk_t = [wc_sb[kc][:, P:2 * P] for kc in range(KC)]
        wv_t = [wc_sb[kc][:, 2 * P:2 * P + C] for kc in range(KC)]
        bqk_sb = pw.tile([P, 2], F32, tag="bqk")
        nc.sync.dma_start(bqk_sb[:], bqk[:])
        bq_sb = bqk_sb[:, 0:1]
        bk_sb = bqk_sb[:, 1:2]
        bv_row = pw.tile([1, C], F32, tag="bvr")
        nc.sync.dma_start(bv_row[:], bvr[:])
        ones_row = pw.tile([1, P], BF16, tag="ones_row")
        nc.vector.memset(ones_row[:], 1.0)
        ones_col = pw.tile([P, 1], F16, tag="ones_col")
        nc.vector.memset(ones_col[:], 1.0)
        ones_row_f = pw.tile([1, P], F32, tag="ones_row_f")
        nc.vector.memset(ones_row_f[:], 1.0)

        # ---- inputs: the first half of each [128, 4096] block arrives as
        # two quarter DMAs (earlier projection start), the rest as halves;
        # both kc blocks interleave so each consumer unblocks earliest.
        d_sb = [pin.tile([P, HW], BF16, tag=f"d{kc}", name=f"d{kc}")
                for kc in range(KC)]
        r_sb = [pin.tile([P, HW], BF16, tag=f"r{kc}", name=f"r{kc}")
                for kc in range(KC)]
        for src, dst in ((d, d_sb), (r, r_sb)):
            for c0, c1 in ((0, 1024), (1024, 2048), (2048, 4096)):
                for kc in range(KC):
                    nc.sync.dma_start(
                        dst[kc][:, c0:c1],
                        src[kc * P:(kc + 1) * P, c0:c1])

        # bv broadcast to 128 partitions: bvb[p, c] = bv[c] (one K=1 matmul).
        bvp = ps_aux.tile([P, C], F32, tag="aux", name="bvp")
        nc.tensor.matmul(bvp[:], lhsT=ones_row_f[:], rhs=bv_row[:],
                         start=True, stop=True)
        bvb = pw.tile([P, C], F32, tag="bvb")
        nc.vector.tensor_copy(bvb[:], bvp[:])

        # ---- q projection (4x-replicated): q4[32j+o, n] = q[o, n] ---------
        # bias adds ride the scalar engine (idle during projections).
        q4 = pqk.tile([P, NQ], BF16, tag="q4")
        for qh in range(2):
            qp = ps_st.tile([P, 1024], F32, tag="stp", name=f"qp{qh}")
            for sub in range(2):
                n0 = sub * NT
                g0 = qh * 1024 + n0
                for kc in range(KC):
                    nc.tensor.matmul(
                        qp[:, n0:n0 + NT],
                        lhsT=wq_t[kc],
                        rhs=d_sb[kc][:, g0:g0 + NT],
                        start=(kc == 0),
                        stop=(kc == KC - 1),
                    )
            nc.scalar.activation(
                q4[:, qh * 1024:(qh + 1) * 1024], qp[:], AF.Identity,
                bias=bq_sb)

        # ---- k projection (4x-replicated over all 4096 keys) --------------
        k4 = pqk.tile([P, HW], BF16, tag="k4")
        for qtr in range(4):
            kp = ps_st.tile([P, 1024], F32, tag="stp", name=f"kp{qtr}")
            for sub in range(2):
                n0 = sub * NT
                g0 = qtr * 1024 + n0
                for kc in range(KC):
                    nc.tensor.matmul(
                        kp[:, n0:n0 + NT],
                        lhsT=wk_t[kc],
                        rhs=d_sb[kc][:, g0:g0 + NT],
                        start=(kc == 0),
                        stop=(kc == KC - 1),
                    )
            nc.scalar.activation(
                k4[:, qtr * 1024:(qtr + 1) * 1024], kp[:], AF.Identity,
                bias=bk_sb)

        # ---- v^T projection: vt[mt][p, c] = v[c, mt*128 + p] + bv[c] ------
        vt_t = []
        for mt in range(MT):
            vp = ps_aux.tile([P, C], F32, tag="aux", name=f"vp{mt}")
            for kc in range(KC):
                nc.tensor.matmul(
                    vp[:],
                    lhsT=r_sb[kc][:, mt * P:(mt + 1) * P],
                    rhs=wv_t[kc],
                    start=(kc == 0),
                    stop=(kc == KC - 1),
                )
            t = pvt.tile([P, C], BF16, tag=f"vt{mt}")
            nc.vector.tensor_tensor(t[:], vp[:], bvb[:], OP.add)
            vt_t.append(t)

        # ---- main attention loop ------------------------------------------
        acc_t = [None] * N_NT
        se_t = [None] * NTOT
        fc_t = [None] * N_NT     # PSUM accumulators (rotating 2 banks)
        fcs_t = [None] * N_NT    # SBUF copies
        sm_t = [None] * N_NT
        rc_t = [None] * N_NT
        bc_t = [None] * N_NT

        def emit_scores_exp(i):
            nt, g = divmod(i, NG)
            stp = ps_st.tile([P, 1024], F32, tag="stp", name=f"stp{i}")
            n0 = nt * NT
            for j in range(2):
                mt = 2 * g + j
                nc.tensor.matmul(
                    stp[:, j * NT:(j + 1) * NT],
                    lhsT=k4[32 * j:32 * j + 32, mt * P:(mt + 1) * P],
                    rhs=q4[32 * j:32 * j + 32, n0:n0 + NT],
                    start=True,
                    stop=True,
                    tile_position=(32 * j, 0),
                )
            se = pse.tile([P, 1024], BF16, tag="se", name=f"se{i}")
            nc.scalar.activation(se[:], stp[:], AF.Exp)
            se_t[i] = se

        def emit_acc(i):
            nt, g = divmod(i, NG)
            if g == 0:
                acc_t[nt] = pacc.tile([P, 1024], F16, tag="acc",
                                      name=f"acc{nt}")
                nc.vector.tensor_copy(acc_t[nt][:], se_t[i][:])
            else:
                nc.vector.tensor_tensor(acc_t[nt][:], acc_t[nt][:],
                                        se_t[i][:], OP.add)

        def emit_feat(i):
            nt, g = divmod(i, NG)
            if g == 0:
                fc_t[nt] = [
                    ps_feat.tile([P, NT], F32, tag="feat",
                                 name=f"fc{nt}_{c}") for c in range(2)]
            fc = fc_t[nt]
            se = se_t[i]
            for j in range(2):
                mt = 2 * g + j
                sej = se[:, j * NT:(j + 1) * NT]
                first = mt == 0
                last = mt == MT - 1
                nc.tensor.matmul(
                    fc[0][:], lhsT=vt_t[mt][:, 0:P], rhs=sej,
                    start=first, stop=last,
                )
                nc.tensor.matmul(
                    fc[1][:], lhsT=vt_t[mt][:, P:C], rhs=sej,
                    start=first, stop=last,
                )
            se_t[i] = None

        def emit_fc_free(nt):
            # Copy fc out of PSUM right away so the next tile's accumulation
            # can claim the banks without waiting for the normalize chain.
            fcs_t[nt] = []
            for c in range(2):
                t = pfcs.tile([P, NT], F32, tag="fcs", name=f"fcs{nt}_{c}")
                nc.vector.tensor_copy(t[:], fc_t[nt][c][:])
                fcs_t[nt].append(t)

        def emit_fold(nt):
            accf = pacc.tile([P, NT], F16, tag="accf", name=f"accf{nt}")
            acc = acc_t[nt]
            nc.vector.tensor_tensor(
                accf[:], acc[:, 0:NT], acc[:, NT:1024], OP.add)
            acc_t[nt] = accf  # reuse slot to pass to emit_sm

        def emit_sm(nt):
            sm = ps_aux.tile([1, NT], F32, tag="aux", name=f"sm{nt}")
            nc.tensor.matmul(sm[:], lhsT=ones_col[:], rhs=acc_t[nt][:],
                             start=True, stop=True)
            sm_t[nt] = sm

        def emit_recip(nt):
            rc = psmall.tile([1, NT], F32, tag="recip")
            nc.vector.reciprocal_approx_fast(out=rc[:], in_=sm_t[nt][:])
            rc_h = psmall.tile([1, NT], BF16, tag="recip_h")
            nc.vector.tensor_copy(rc_h[:], rc[:])
            rc_t[nt] = rc_h

        def emit_bc(nt):
            bc = ps_aux.tile([P, NT], F32, tag="aux", name=f"bc{nt}")
            nc.tensor.matmul(bc[:], lhsT=ones_row[:], rhs=rc_t[nt][:],
                             start=True, stop=True)
            bc_t[nt] = bc

        def emit_norm_out(nt):
            n0 = nt * NT
            bc_sb = pout.tile([P, NT], F32, tag="bc_sb")
            nc.vector.tensor_copy(bc_sb[:], bc_t[nt][:])
            for c in range(2):
                ot = pout.tile([P, NT], F32, tag="ot")
                nc.vector.tensor_tensor(ot[:], fcs_t[nt][c][:], bc_sb[:],
                                        OP.mult)
                nc.sync.dma_start(out[c * P:(c + 1) * P, n0:n0 + NT], ot[:])

        # Slot schedule: front work for group i, feat lagging TWO groups so
        # the scores->exp->feat dependency round-trip spans three slots and
        # the loop is paced by pure PE work, not the latency chain. Tile
        # tails are staggered so no PE instruction waits on a long DVE chain.
        for i in range(NTOT + 4):
            if i < NTOT:
                emit_scores_exp(i)
            if 2 <= i <= NTOT + 1:
                emit_feat(i - 2)
                if (i - 2) % NG == NG - 1:
                    emit_fc_free((i - 2) // NG)
                    emit_fold((i - 2) // NG)
            if i < NTOT:
                emit_acc(i)
            # tails for tile nt run at slots nt*NG+NG+2 (sm) / +3 (bc, norm)
            if i >= NG + 2 and (i - NG - 2) % NG == 0:
                nt = (i - NG - 2) // NG
                emit_sm(nt)
                emit_recip(nt)
            if i >= NG + 3 and (i - NG - 3) % NG == 0:
                nt = (i - NG - 3) // NG
                emit_bc(nt)
                emit_norm_out(nt)


_BUILT = None


def _build():
    global _BUILT
    if _BUILT is not None:
        return _BUILT
    nc = bacc.Bacc("TRN2", target_bir_lowering=False, debug=False)
    io = {
        "d": nc.dram_tensor("d", [C, HW], BF16, kind="ExternalInput"),
        "r": nc.dram_tensor("r", [C, HW], BF16, kind="ExternalInput"),
        "wcat": nc.dram_tensor("wcat", [C, 512], BF16, kind="ExternalInput"),
        "bqk": nc.dram_tensor("bqk", [P, 2], F32, kind="ExternalInput"),
        "bvr": nc.dram_tensor("bvr", [1, C], F32, kind="ExternalInput"),
        "out": nc.dram_tensor("out", [C, NQ], F32, kind="ExternalOutput"),
    }
    with tile.TileContext(nc) as tc:
        _emit(tc, io)
    nc.compile()
    _BUILT = nc
    return nc


def _in_maps(rgb, depth, Wq, bq, Wk, bk, Wv, bv):
    f = np.float32
    d_all = np.asarray(depth, f).reshape(B, C, HW).astype(BF16_NP)
    r_all = np.asarray(rgb, f).reshape(B, C, HW).astype(BF16_NP)
    wqt4 = np.tile(np.asarray(Wq, f).T, (1, 4))
    wkt4 = np.tile(np.asarray(Wk, f).T, (1, 4))
    wvt = np.asarray(Wv, f).T
    wcat = np.ascontiguousarray(
        np.concatenate([wqt4, wkt4, wvt], axis=1).astype(BF16_NP))
    bqk = np.ascontiguousarray(
        np.stack([np.tile(np.asarray(bq, f), 4),
                  np.tile(np.asarray(bk, f), 4)], axis=1))
    bvr = np.ascontiguousarray(np.asarray(bv, f).reshape(1, C))
    maps = []
    for core in range(8):
        b, half = core // 2, core % 2
        # Rotate the key axis so this core's query half sits at cols 0:2048;
        # softmax + the value reduction are permutation-invariant over keys
        # as long as d and r use the same rotation.
        rot = np.r_[half * NQ:(half * NQ + HW)] % HW
        maps.append({
            "d": np.ascontiguousarray(d_all[b][:, rot]),
            "r": np.ascontiguousarray(r_all[b][:, rot]),
            "wcat": wcat, "bqk": bqk, "bvr": bvr,
        })
    return maps


def kernel(rgb, depth, Wq, bq, Wk, bk, Wv, bv, **run_kwargs):
    nc = _build()
    maps = _in_maps(rgb, depth, Wq, bq, Wk, bk, Wv, bv)
    res = run_bass_kernel_spmd(nc, maps, core_ids=list(range(8)), **run_kwargs)
    results = res.results if hasattr(res, "results") else res
    out = np.empty((B, C, HW), dtype=np.float32)
    for core in range(8):
        b, half = core // 2, core % 2
        out[b][:, half * NQ:(half + 1) * NQ] = results[core]["out"]
    kernel.last_results = res
    return out.reshape(B, C, H, W)
